# revision 1
# baseline (speedup 1.0000x reference)
"""Causal Conv1d (K=4) + bias + silu for TRN2, sharded over 8 NeuronCores.

Reference op: x (B=4, C_IN=2048, S=4096) fp32, weight (C_OUT=2048, C_IN, 4),
bias (C_OUT,);  out = silu(causal_conv1d(x, weight) + bias).

Sharding: data-parallel over sequence. Core c computes out[:, :, c*512:(c+1)*512]
from x[:, :, c*512-3 : c*512+512] (zero-padded left halo), full weight/bias.

Per-core compute: the conv is 16 k-tiles x 4 taps = 64 accumulating
128x128x512 matmuls per (batch, m-tile) PSUM group. Tap t uses a shifted
free-dim view of the resident x tile - no shifted copies are materialized.
Matmuls accumulate in fp32 PSUM. Weights are pre-transposed on the host to
(mi, p, ki, t, f) so each per-m-tile weight chunk is one DMA with
long-contiguous per-partition lines. The 4 batches accumulate into 4 PSUM
banks in parallel so each weight tile's uses are back-to-back and the next
m-tile (4 more banks) overlaps eviction (silu+bias on ScalarE) and store.
"""

import numpy as np

import concourse.bacc as bacc
import concourse.bass as bass
import concourse.mybir as mybir
import concourse.tile as tile
from concourse.bass_utils import run_bass_kernel_spmd

P = 128

# Problem constants (hardcoded per harness contract).
B = 4
C_IN = 2048
C_OUT = 2048
KTAPS = 4
S = 4096
N_CORES = 8
S_CHUNK = S // N_CORES          # 512
HALO = KTAPS - 1                # 3

# Matmul operand dtype: "f32r" (FP22-truncated fp32, rel err ~1.6e-4) or
# "f16" (fp16 operands, rel err ~1e-3). Both measured ~1.2 ms/pass on HW;
# f32r kept for precision. Weight stream on HWDGE: 1.098 ms/pass measured.
MM_DTYPE = "f32r"


def build_conv_nc(
    b, n_ki, n_mi, ktaps, s_chunk, ki_per_wchunk, act_fn=None, reps=1,
    mm_dtype=None,
):
    """Build the per-core Bass program.

    b:        batches
    n_ki:     C_IN / 128 contraction tiles
    n_mi:     C_OUT / 128 output tiles
    ktaps:    conv taps
    s_chunk:  output sequence columns per core
    ki_per_wchunk: k-tiles per weight DMA chunk
    """
    halo = ktaps - 1
    s_in = s_chunk + halo
    assert n_ki % ki_per_wchunk == 0
    n_wchunks = n_ki // ki_per_wchunk
    wchunk_cols = ki_per_wchunk * ktaps * P

    mm_dtype = mm_dtype or MM_DTYPE
    if mm_dtype == "f32r":
        in_dt = mybir.dt.float32      # dram dtype of x/w
        mm_dt = mybir.dt.float32r     # sbuf tile dtype fed to the PE
        cast_dma = False              # fp32->fp32r is a bitcast
    elif mm_dtype == "f16":
        in_dt = mybir.dt.float16      # host pre-converts x/w to fp16
        mm_dt = mybir.dt.float16
        cast_dma = False
    else:
        raise ValueError(mm_dtype)

    # Bacc (not raw Bass): its compile() splits multi-wait instructions into
    # event-semaphore sequences and moves matmul waits onto ldweights —
    # without it, walrus rejects any instruction carrying >1 sync wait.
    nc = bacc.Bacc("TRN2", target_bir_lowering=False, debug=False)

    x_d = nc.dram_tensor(
        "x", [b, n_ki * P, s_in], in_dt, kind="ExternalInput"
    ).ap()
    # weight pre-layout: (mi, p, ki, t, f) = W[mi*128+f, ki*128+p, t]
    w_d = nc.dram_tensor(
        "w", [n_mi, P, n_ki, ktaps, P], in_dt, kind="ExternalInput"
    ).ap()
    # bias pre-layout: [p, mi] = bias[mi*128+p]
    bias_d = nc.dram_tensor(
        "bias", [P, n_mi], mybir.dt.float32, kind="ExternalInput"
    ).ap()
    out_d = nc.dram_tensor(
        "out", [b, n_mi * P, s_chunk], mybir.dt.float32, kind="ExternalOutput"
    ).ap()

    f32 = mybir.dt.float32
    silu = act_fn if act_fn is not None else mybir.ActivationFunctionType.Silu

    def dma_src(ap):
        # fp32r tiles must be *written* as fp32r (BIR verifier rule);
        # the DRAM side carries the same bits, so bitcast the source.
        return ap.bitcast(mm_dt) if mm_dt != in_dt else ap

    # Raw PSUM banks, manually rotated (mi parity picks the half). Raw
    # tensors get full RAW/WAR/WAW tracking from TileContext's shadow
    # memory but none of the pool's slot-release waits.
    ps_banks = [
        nc.alloc_psum_tensor(f"psb{k}", [P, s_chunk], mybir.dt.float32).ap()
        for k in range(2 * b)
    ]

    with tile.TileContext(nc) as tc:
        with (
            tc.tile_pool(name="xpool", bufs=1) as xpool,
            tc.tile_pool(name="wpool", bufs=2) as wpool,
            tc.tile_pool(name="bpool", bufs=1) as bpool,
            tc.tile_pool(name="opool", bufs=4) as opool,
        ):
            bias_t = bpool.tile([P, n_mi], f32, tag="bias")
            nc.sync.dma_start(out=bias_t, in_=bias_d)

            # Resident x tiles: one [128, s_in] tile per (batch, k-tile).
            x_t = {}
            for bi in range(b):
                for ki in range(n_ki):
                    t_ = xpool.tile([P, s_in], mm_dt, tag=f"x{bi}_{ki}")
                    nc.sync.dma_start(
                        out=t_,
                        in_=dma_src(x_d[bi, ki * P : (ki + 1) * P, :]),
                    )
                    x_t[bi, ki] = t_

            for rep in range(reps):
              for mi in range(n_mi):
                psums = [
                    ps_banks[((rep * n_mi + mi) % 2) * b + bi]
                    for bi in range(b)
                ]
                for c in range(n_wchunks):
                    w_t = wpool.tile([P, wchunk_cols], mm_dt, tag="w")
                    nc.sync.dma_start(
                        out=w_t,
                        in_=dma_src(
                            w_d[
                                mi, :,
                                c * ki_per_wchunk : (c + 1) * ki_per_wchunk,
                                :, :,
                            ]
                        ),
                    )
                    for kic in range(ki_per_wchunk):
                        ki = c * ki_per_wchunk + kic
                        for t in range(ktaps):
                            col0 = (kic * ktaps + t) * P
                            lhsT = w_t[:, col0 : col0 + P]
                            first = ki == 0 and t == 0
                            last = ki == n_ki - 1 and t == ktaps - 1
                            for bi in range(b):
                                rhs = x_t[bi, ki][:, t : t + s_chunk]
                                nc.tensor.matmul(
                                    psums[bi], lhsT, rhs, start=first, stop=last
                                )
                for bi in range(b):
                    o_t = opool.tile([P, s_chunk], f32, tag="o")
                    nc.scalar.activation(
                        o_t, psums[bi], silu, bias=bias_t[:, mi : mi + 1]
                    )
                    nc.sync.dma_start(
                        out=out_d[bi, mi * P : (mi + 1) * P, :], in_=o_t
                    )
    nc.compile()
    return nc


def prep_weight(weight, n_mi, n_ki, ktaps):
    # (C_OUT, C_IN, K) -> (mi, p, ki, t, f) with o=(mi,f), i=(ki,p)
    w = weight.reshape(n_mi, P, n_ki, P, ktaps)  # (mi, f, ki, p, t)
    return np.ascontiguousarray(w.transpose(0, 3, 2, 4, 1))


def host_in_dtype(mm_dtype=None):
    mm_dtype = mm_dtype or MM_DTYPE
    if mm_dtype == "f16":
        return np.float16
    return np.float32


def kernel(x, weight, bias):
    x = np.asarray(x, dtype=np.float32)
    weight = np.asarray(weight, dtype=np.float32)
    bias = np.asarray(bias, dtype=np.float32)

    n_ki = C_IN // P
    n_mi = C_OUT // P
    hdt = host_in_dtype()

    xp = np.pad(x, ((0, 0), (0, 0), (HALO, 0)))  # (B, C_IN, S+3)
    w3 = prep_weight(weight, n_mi, n_ki, KTAPS).astype(hdt)
    bias2 = np.ascontiguousarray(bias.reshape(n_mi, P).T)  # (P, n_mi)

    nc = build_conv_nc(B, n_ki, n_mi, KTAPS, S_CHUNK, ki_per_wchunk=8)

    in_maps = []
    for c in range(N_CORES):
        xc = np.ascontiguousarray(
            xp[:, :, c * S_CHUNK : c * S_CHUNK + S_CHUNK + HALO]
        ).astype(hdt)
        in_maps.append({"x": xc, "w": w3, "bias": bias2})

    global LAST_RESULT
    res = run_bass_kernel_spmd(
        nc, in_maps, core_ids=list(range(N_CORES)), trace=PROFILE
    )
    LAST_RESULT = res
    out = np.concatenate([r["out"] for r in res.results], axis=2)
    return out


PROFILE = False
LAST_RESULT = None



# revision 10
# speedup vs baseline: 2.0907x; 2.0907x over previous
"""Causal Conv1d (K=4) + bias + silu for TRN2 via Winograd F(4,4), 8 cores.

Reference op: x (B=4, C_IN=2048, S=4096) fp32, weight (C_OUT=2048, C_IN, 4),
bias (C_OUT,);  out = silu(causal_conv1d(x, weight) + bias).

Sharding: data-parallel over sequence; core c computes out[:, :, c*512:(c+1)*512]
from x[:, :, c*512-3 : c*512+512] (zero-padded left halo), full weight/bias.

Algorithm: Winograd/Toom-Cook F(4,4) over the sequence dim with 7 finite
points {0, +-1, +-2, +-1/2}. Each tile of m=4 outputs needs U=7 transformed
products instead of 16 direct MACs: PE work drops to 7/16 of direct conv
(1792 accumulating 128x128x512 fp16 matmuls per core instead of 4096).

  y = A^T [ (G w) . (B^T d) ]   per tile of 4 outputs, 7-point window d
  B^T rows = Lagrange numerator polys (dense, evaluated on DVE with
             even/odd CSE: 21 fp16 ops per k-tile)
  A^T      = Vandermonde powers (incremental S/D combine on DVE, fp16)
  G w      = host-precomputed fp16 weights (streamed, 7/4 x direct size @fp16)

Pipeline: 3 stages over point-pairs {+-1}, {+-2}, {+-1/2, 0}. Per stage the
DVE transforms all 16 k-tiles (BX, fp16), then PE accumulates per (mi, u)
PSUM groups which DVE combines (Vandermonde) into per-mi fp16 y tiles.
Stages of mi-loop matmuls overlap the next stage's input transform. Act does
the final strided-gather silu+bias into fp32 out tiles.

Numerics: x/BX/GW/y in fp16, PSUM fp32. Measured model error ~4e-3 of
max|out| (gate 2e-2); fp16 matmul runs at the same PE rate as f32r.
"""

import numpy as np

import concourse.bacc as bacc
import concourse.bass as bass
import concourse.mybir as mybir
import concourse.tile as tile
from concourse.alu_op_type import AluOpType
from concourse.bass_utils import run_bass_kernel_spmd

P = 128

B = 4
C_IN = 2048
C_OUT = 2048
KTAPS = 4
S = 4096
N_CORES = 8
S_CHUNK = S // N_CORES          # 512
HALO = KTAPS - 1                # 3

M_TILE = 4                       # F(4,4): outputs per winograd tile
U = 7                            # transform size
NT = S_CHUNK // M_TILE           # 128 winograd tiles per batch
FD = B * NT                      # 512 matmul free dim (4 batches side by side)
N_KI = C_IN // P                 # 16
N_MI = C_OUT // P                # 16

# Point order in weight/BX memory: [+1, -1, +1/2, -1/2, +2, -2, 0]
# (stage grouping: stage0={+1,-1}, stage1={+1/2,-1/2}, stage2={+2,-2,0})
POINTS_MEM = [1.0, -1.0, 0.5, -0.5, 2.0, -2.0, 0.0]
STAGES = [(0, 2), (2, 4), (4, 7)]   # u-ranges per stage


def winograd_G():
    """G (U x r) for points POINTS_MEM (last = 0), r=4, exact rationals."""
    from fractions import Fraction as F
    pts = [F(1), F(-1), F(1, 2), F(-1, 2), F(2), F(-2), F(0)]
    G = []
    for u, p in enumerate(pts):
        den = F(1)
        for k, q in enumerate(pts):
            if k != u:
                den *= (p - q)
        G.append([p ** t / den for t in range(KTAPS)])
    return np.array([[float(c) for c in row] for row in G])


def build_winograd_nc(reps=1, schedule="fused"):
    f16 = mybir.dt.float16
    f32 = mybir.dt.float32
    MUL, ADD, SUB = AluOpType.mult, AluOpType.add, AluOpType.subtract
    silu_fn = mybir.ActivationFunctionType.Silu

    nc = bacc.Bacc("TRN2", target_bir_lowering=False, debug=False)

    # x planes: [ki, p, v(7), bi(4), t(128)] fp16; plane v holds x_pad[4t+v]
    xp_d = nc.dram_tensor(
        "x", [N_KI, P, U, B, NT], f16, kind="ExternalInput"
    ).ap()
    # winograd weights: [mi, p(ci), u, ki, f(co)] fp16
    w_d = nc.dram_tensor(
        "w", [N_MI, P, U, N_KI, P], f16, kind="ExternalInput"
    ).ap()
    bias_d = nc.dram_tensor("bias", [P, N_MI], f32, kind="ExternalInput").ap()
    out_d = nc.dram_tensor(
        "out", [B, N_MI * P, S_CHUNK], f32, kind="ExternalOutput"
    ).ap()

    ps_banks = [
        nc.alloc_psum_tensor(f"psb{k}", [P, FD], f32).ap() for k in range(8)
    ]
    bank_ctr = [0]

    def next_bank():
        b = ps_banks[bank_ctr[0] % 8]
        bank_ctr[0] += 1
        return b

    def stt(out, in0, scalar, in1):
        nc.vector.scalar_tensor_tensor(out, in0, scalar, in1, MUL, ADD)

    def tt(out, in0, in1, op):
        nc.vector.tensor_tensor(out, in0, in1, op)

    with tile.TileContext(nc) as tc:
        with (
            tc.tile_pool(name="plpool", bufs=3) as plpool,
            tc.tile_pool(name="bxpool", bufs=2) as bxpool,
            tc.tile_pool(name="bzpool", bufs=1) as bzpool,
            tc.tile_pool(name="ypool", bufs=1) as ypool,
            tc.tile_pool(name="wpool", bufs=3) as wpool,
            tc.tile_pool(name="opool", bufs=4) as opool,
            tc.tile_pool(name="scpool", bufs=2) as scpool,
            tc.tile_pool(name="bpool", bufs=1) as bpool,
        ):
            bias_t = bpool.tile([P, N_MI], f32, tag="bias")
            nc.sync.dma_start(out=bias_t, in_=bias_d)

            for rep in range(reps):
                y_t = {}
                for mi in range(N_MI):
                    y_t[mi] = ypool.tile([P, M_TILE * FD], f16, tag=f"y{mi}", name=f"y{mi}")

                # bx[(stage, ki, uu)] -> AP of transformed input
                bx = {}

                def emit_transform(sts, ki):
                    pl = plpool.tile([P, U * FD], f16, tag="pl", name="pl")
                    nc.sync.dma_start(out=pl, in_=xp_d[ki])
                    d = [pl[:, v * FD:(v + 1) * FD] for v in range(U)]
                    for st in sts:
                        bx_t = bxpool.tile(
                            [P, 2 * FD], f16, tag=f"bx{ki}", name=f"bx{ki}"
                        )
                        ep = bx_t[:, 0:FD]
                        em = bx_t[:, FD:2 * FD]
                        o = scpool.tile([P, FD], f16, tag="osc", name="osc")
                        if st == 0:         # points +-1
                            stt(ep, d[4], -4.25, d[2])
                            tt(ep, ep, d[6], ADD)
                            stt(o, d[3], -4.25, d[1])
                            tt(o, o, d[5], ADD)
                            tt(em, ep, o, SUB)
                            tt(ep, ep, o, ADD)
                        elif st == 1:       # points +-1/2
                            stt(ep, d[2], 4.0, d[6])
                            stt(ep, d[4], -5.0, ep)
                            stt(o, d[3], -5.0, d[5])
                            stt(o, d[1], 4.0, o)
                            stt(em, o, -0.5, ep)
                            stt(ep, o, 0.5, ep)
                        else:               # points +-2 and 0
                            stt(ep, d[2], 0.25, d[6])
                            stt(ep, d[4], -1.25, ep)
                            stt(o, d[3], -5.0, d[1])
                            stt(o, d[5], 4.0, o)
                            stt(em, o, -0.5, ep)
                            stt(ep, o, 0.5, ep)
                            bz = bzpool.tile(
                                [P, FD], f16, tag=f"bz{ki}", name=f"bz{ki}"
                            )
                            tt(bz, d[6], d[0], SUB)
                            o2 = scpool.tile([P, FD], f16, tag="osc", name="osc")
                            tt(o2, d[2], d[4], SUB)
                            stt(bz, o2, 5.25, bz)
                            bx[st, ki, 2] = bz
                        bx[st, ki, 0] = bx_t[:, 0:FD]
                        bx[st, ki, 1] = bx_t[:, FD:2 * FD]

                def emit_migroup(stage, mi):
                    u_lo, u_hi = STAGES[stage]
                    n_u = u_hi - u_lo
                    if True:
                        psums = []
                        for uu in range(n_u):
                            w_t = wpool.tile([P, N_KI * P], f16, tag="w", name="w")
                            nc.sync.dma_start(
                                out=w_t, in_=w_d[mi, :, u_lo + uu, :, :]
                            )
                            ps = next_bank()
                            psums.append(ps)
                            for ki in range(N_KI):
                                lhsT = w_t[:, ki * P:(ki + 1) * P]
                                nc.tensor.matmul(
                                    ps, lhsT, bx[stage, ki, uu],
                                    start=(ki == 0), stop=(ki == N_KI - 1),
                                )
                        # Act evicts each closed PSUM group to fp16 SBUF
                        # (walrus: only one PSUM operand per DVE op)
                        mts = []
                        for uu in range(n_u):
                            m_t = scpool.tile([P, FD], f16, tag=f"m{uu}")
                            nc.scalar.copy(m_t, psums[uu])
                            mts.append(m_t)
                        y = y_t[mi]
                        yj = [y[:, j * FD:(j + 1) * FD] for j in range(M_TILE)]
                        pa, pb = mts[0], mts[1]
                        if stage == 0:
                            tt(yj[0], pa, pb, ADD)
                            tt(yj[1], pa, pb, SUB)
                            nc.vector.tensor_copy(yj[2], yj[0])
                            nc.vector.tensor_copy(yj[3], yj[1])
                        elif stage == 1:
                            s2 = scpool.tile([P, FD], f16, tag="s2")
                            d2 = scpool.tile([P, FD], f16, tag="d2")
                            tt(s2, pa, pb, ADD)
                            tt(d2, pa, pb, SUB)
                            tt(yj[0], s2, yj[0], ADD)
                            stt(yj[2], s2, 0.25, yj[2])
                            stt(yj[1], d2, 0.5, yj[1])
                            stt(yj[3], d2, 0.125, yj[3])
                        else:
                            sh = scpool.tile([P, FD], f16, tag="s2")
                            dh = scpool.tile([P, FD], f16, tag="d2")
                            tt(sh, pa, pb, ADD)
                            tt(dh, pa, pb, SUB)
                            tt(yj[0], sh, yj[0], ADD)
                            tt(yj[0], mts[2], yj[0], ADD)
                            stt(yj[2], sh, 4.0, yj[2])
                            stt(yj[1], dh, 2.0, yj[1])
                            stt(yj[3], dh, 8.0, yj[3])
                            # finished: silu + bias, de-interleave, store
                            for bi in range(B):
                                o_t = opool.tile([P, S_CHUNK], f32, tag="o")
                                for j in range(M_TILE):
                                    nc.scalar.activation(
                                        o_t[:, j:S_CHUNK:M_TILE],
                                        y[:, j * FD + bi * NT:
                                           j * FD + (bi + 1) * NT],
                                        silu_fn,
                                        bias=bias_t[:, mi:mi + 1],
                                    )
                                nc.sync.dma_start(
                                    out=out_d[bi, mi * P:(mi + 1) * P, :],
                                    in_=o_t,
                                )

                if schedule == "fused":
                    # one plane load feeds stage0+stage1 transforms up
                    # front; stage2 transforms interleave into stage1's
                    # matmul loop (PE never waits at stage boundaries,
                    # plane DMA read twice per pass).
                    for ki in range(N_KI):
                        emit_transform([0, 1], ki)
                    for mi in range(N_MI):
                        emit_migroup(0, mi)
                    for mi in range(N_MI):
                        emit_migroup(1, mi)
                        emit_transform([2], mi)
                    for mi in range(N_MI):
                        emit_migroup(2, mi)
                else:
                    # spread: only stage0 transforms up front (shortest
                    # PE fill); stage s+1 transforms interleave into
                    # stage s's matmul loop (plane DMA read 3x per pass).
                    for ki in range(N_KI):
                        emit_transform([0], ki)
                    for mi in range(N_MI):
                        emit_migroup(0, mi)
                        emit_transform([1], mi)
                    for mi in range(N_MI):
                        emit_migroup(1, mi)
                        emit_transform([2], mi)
                    for mi in range(N_MI):
                        emit_migroup(2, mi)
    nc.compile()
    return nc


def prep_inputs(x, weight, bias):
    """Full fp32 inputs -> per-core in_maps with winograd host transforms."""
    x = np.asarray(x, dtype=np.float32)
    weight = np.asarray(weight, dtype=np.float32)
    bias = np.asarray(bias, dtype=np.float32)

    G = winograd_G()
    # GW[u, co, ci] fp16, laid out [mi, p(ci), u, ki, f(co)]
    GW = np.einsum('ut,oit->uoi', G, weight).astype(np.float16)
    GW = GW.reshape(U, N_MI, P, N_KI, P)          # (u, mi, f, ki, p)
    w_host = np.ascontiguousarray(GW.transpose(1, 4, 0, 3, 2))

    bias2 = np.ascontiguousarray(bias.reshape(N_MI, P).T)  # (P, n_mi)

    xp = np.pad(x, ((0, 0), (0, 0), (HALO, 0))).astype(np.float16)  # (B,CI,S+3)
    in_maps = []
    for c in range(N_CORES):
        xc = xp[:, :, c * S_CHUNK: c * S_CHUNK + S_CHUNK + HALO]  # (B,CI,515)
        # planes[v][t] = xc[..., 4t+v], t<128 -> layout [ki,p,v,bi,t]
        pl = np.empty((B, C_IN, U, NT), dtype=np.float16)
        for v in range(U):
            pl[:, :, v, :] = xc[:, :, v: v + 4 * NT: 4][:, :, :NT]
        pl = pl.reshape(B, N_KI, P, U, NT)
        pl = np.ascontiguousarray(pl.transpose(1, 2, 3, 0, 4))  # ki,p,v,bi,t
        in_maps.append({"x": pl, "w": w_host, "bias": bias2})
    return in_maps


def build_for_bench(x, weight, bias, reps=1):
    nc = build_winograd_nc(reps=reps)
    in_maps = prep_inputs(x, weight, bias)
    return nc, in_maps


def kernel(x, weight, bias):
    nc, in_maps = build_for_bench(x, weight, bias, reps=1)
    global LAST_RESULT
    res = run_bass_kernel_spmd(
        nc, in_maps, core_ids=list(range(N_CORES)), trace=PROFILE
    )
    LAST_RESULT = res
    out = np.concatenate([r["out"] for r in res.results], axis=2)
    return out


PROFILE = False
LAST_RESULT = None
